# revision 18
# baseline (speedup 1.0000x reference)
"""Trainium2 Bass kernel for quantized int8 Conv2d (dequant -> conv -> bias).

Reference computation (per-tensor quant):
    x = (inputVec.f32 - 7) * 0.01        # [32, 128, 56, 56]
    w = (weight.f32   - 3) * 0.01        # [256, 128, 3, 3]
    b = clip(round(bias / 1e-4)) * 1e-4  # [256]
    out = conv2d(x, w, VALID) + b        # [32, 256, 54, 54] f32

Strategy:
  - Data-parallel over batch: 32 images -> 8 cores x 4 images.
  - Integer arithmetic is exact in bf16/fp32: (w-3) in [-131,124] and x_q in
    [-128,127] are exactly representable in bf16; the 1152-term dot products
    stay well inside fp32's exact-integer range in practice.
  - Because padding is VALID, every output pixel sees the full 3x3x128 window,
    so the input zero-point (-7) contribution is a per-channel constant:
        sum (x-7)(w-3) = sum x*(w-3) - 7 * S3[co],  S3[co] = sum (w-3)
    We fold -7e-4*S3 plus the quantized bias into one per-channel bias on host.
  - On device: conv as 9 shifted matmuls (kh,kw) accumulating in PSUM.
    lhsT = (w-3)[ci, co_chunk] bf16 (128x128), rhs = x_bf16[ci, 9 rows x 54
    cols strided view] (N=486 <= 512 one PSUM bank), out chunks of 128 Cout.
  - Epilogue on ScalarE: out = psum * 1e-4 + bias_eff (per-partition bias AP),
    then DMA to DRAM.
"""

import os
import sys

import numpy as np

if "/opt/trn_rl_repo" not in sys.path:
    sys.path.insert(0, "/opt/trn_rl_repo")

import ml_dtypes

import concourse.bass as bass
import concourse.mybir as mybir
from concourse import bacc
from concourse.tile import TileContext

# Problem constants (hardcoded per harness contract)
N_FULL = 32
CIN = 128
H = 56
W = 56
COUT = 256
KH = 3
KW = 3
HO = H - 2  # 54
WO = W - 2  # 54
N_CORES = 8
N_PER_CORE = N_FULL // N_CORES  # 4

IN_SCALE, IN_ZP = 0.01, 7
W_SCALE, W_ZP = 0.01, 3
B_SCALE = 1e-4
OUT_SCALE = IN_SCALE * W_SCALE  # 1e-4

ROWS_PER_TILE = 9           # output rows per matmul group
N_ROW_TILES = HO // ROWS_PER_TILE  # 6
NFREE = ROWS_PER_TILE * WO  # 486 <= 512 (one PSUM bank)


def build_nc():
    nc = bacc.Bacc()

    x = nc.declare_dram_parameter("x", [N_PER_CORE, CIN, H * W], mybir.dt.int8, isOutput=False)
    wT = nc.declare_dram_parameter("wT", [KH * KW, CIN, COUT], mybir.dt.bfloat16, isOutput=False)
    b = nc.declare_dram_parameter("b", [CIN, 2], mybir.dt.float32, isOutput=False)
    out = nc.declare_dram_parameter("out", [N_PER_CORE, 2, 128, HO * WO], mybir.dt.float32, isOutput=True)

    with TileContext(nc) as tc:
        with (
            tc.tile_pool(name="const", bufs=1) as const_pool,
            tc.tile_pool(name="xq", bufs=4) as xq_pool,
            tc.tile_pool(name="xb", bufs=4) as xb_pool,
            tc.tile_pool(name="psum", bufs=6, space="PSUM") as psum_pool,
            tc.tile_pool(name="osb", bufs=8) as out_pool,
        ):
            # Input images on the SP HWDGE queue; weights/bias on the ACT
            # HWDGE queue so they don't serialize in front of image 0.
            xqs, xbs = [], []
            for n in range(N_PER_CORE):
                xq = xq_pool.tile([CIN, H * W], mybir.dt.int8, name=f"xq{n}", tag="xq")
                nc.sync.dma_start(out=xq[:], in_=x[n])
                xqs.append(xq)

            # Weights: [ci, (k, co)] bf16, all 9 taps resident.
            w_sb = const_pool.tile([CIN, KH * KW * COUT], mybir.dt.bfloat16)
            for k in range(KH * KW):
                nc.scalar.dma_start(out=w_sb[:, k * COUT:(k + 1) * COUT], in_=wT[k])
            # Effective bias: [co_within_chunk, chunk] f32
            b_sb = const_pool.tile([128, 2], mybir.dt.float32)
            nc.scalar.dma_start(out=b_sb[:], in_=b[:])

            for n in range(N_PER_CORE):
                xb = xb_pool.tile([CIN, H * W], mybir.dt.bfloat16, name=f"xb{n}", tag="xb")
                nc.vector.tensor_copy(xb[:], xqs[n][:])
                xbs.append(xb)

            for n in range(N_PER_CORE):
                xb3 = xbs[n].rearrange("p (h w) -> p h w", w=W)

                for chunk in range(2):
                    ot = out_pool.tile([128, HO * WO], mybir.dt.float32)
                    for rt in range(N_ROW_TILES):
                        ps = psum_pool.tile([128, NFREE], mybir.dt.float32)
                        y0 = rt * ROWS_PER_TILE
                        for k in range(KH * KW):
                            kh, kw = divmod(k, KW)
                            lhsT = w_sb[:, k * COUT + chunk * 128:k * COUT + chunk * 128 + 128]
                            rhs = xb3[:, y0 + kh:y0 + kh + ROWS_PER_TILE, kw:kw + WO]
                            nc.tensor.matmul(
                                ps[:], lhsT=lhsT, rhs=rhs,
                                start=(k == 0), stop=(k == KH * KW - 1),
                            )
                        nc.scalar.activation(
                            ot[:, y0 * WO:(y0 + ROWS_PER_TILE) * WO], ps[:],
                            mybir.ActivationFunctionType.Identity,
                            bias=b_sb[:, chunk:chunk + 1], scale=float(OUT_SCALE),
                        )
                        if rt % 2 == 1:  # store every 2 row tiles for overlap
                            c0 = (y0 - ROWS_PER_TILE) * WO
                            c1 = (y0 + ROWS_PER_TILE) * WO
                            nc.sync.dma_start(
                                out=out[n, chunk, :, c0:c1], in_=ot[:, c0:c1]
                            )
    nc.compile()
    return nc


def host_prep(weight: np.ndarray, bias: np.ndarray):
    """Prepare the replicated weight/bias operands (layout + zero-point folding)."""
    w_int = weight.astype(np.int32) - W_ZP  # [256, 128, 3, 3], in [-131, 124]
    # lhsT layout: [kh*kw, ci, co]
    wT = np.ascontiguousarray(w_int.transpose(2, 3, 1, 0).reshape(KH * KW, CIN, COUT))
    wT_bf16 = wT.astype(ml_dtypes.bfloat16)  # exact (|v| <= 131 < 256)

    # Quantized bias, matching reference rounding (f32 ops, round-half-even)
    b_q = np.clip(np.round(bias.astype(np.float32) / np.float32(B_SCALE)),
                  -(2.0 ** 31), 2.0 ** 31 - 1).astype(np.float32) * np.float32(B_SCALE)
    s3 = w_int.reshape(COUT, -1).sum(axis=1).astype(np.float64)  # sum (w-3) per co
    b_eff = (b_q.astype(np.float64) - IN_ZP * OUT_SCALE * s3).astype(np.float32)
    # [co_within_chunk, chunk] so the SBUF partition dim is contiguous in DRAM
    b_dev = np.ascontiguousarray(b_eff.reshape(2, 128).T)
    return wT_bf16, b_dev


_CACHED_NC = None


def _install_trace_hook():
    """Register the axon NTFF profile hook that this container's antenv stub
    lacks, and neutralize the remote artifact upload. Best-effort."""
    try:
        import types

        import antenv  # noqa: F401

        if "antenv.axon_hooks" not in sys.modules:
            from trn_agent_boot.trn_boot import _ntff_profile_via_ctypes

            hook = _ntff_profile_via_ctypes("/opt/axon/libaxon_pjrt.so")
            mod = types.ModuleType("antenv.axon_hooks")
            mod.get_axon_ntff_profile_hook = lambda: hook
            mod.set_axon_ntff_profile_hook = lambda h: None
            sys.modules["antenv.axon_hooks"] = mod
        import concourse.bass_utils as bu

        bu.upload_artifacts = lambda tmpdir: str(tmpdir)
        return True
    except Exception as e:  # pragma: no cover - profiling is best-effort
        print(f"trace hook unavailable: {type(e).__name__}: {e}", file=sys.stderr)
        return False


def kernel(inputVec: np.ndarray, weight: np.ndarray, bias: np.ndarray) -> np.ndarray:
    global _CACHED_NC
    from concourse.bass_utils import run_bass_kernel_spmd

    wT_bf16, b_dev = host_prep(weight, bias)
    x = np.ascontiguousarray(inputVec.reshape(N_FULL, CIN, H * W))

    if _CACHED_NC is None:
        _CACHED_NC = build_nc()
    nc = _CACHED_NC

    in_maps = []
    for c in range(N_CORES):
        in_maps.append({
            "x": x[c * N_PER_CORE:(c + 1) * N_PER_CORE],
            "wT": wT_bf16,
            "b": b_dev,
        })

    trace = os.environ.get("KERNEL_TRACE", "0") == "1"
    if trace:
        trace = _install_trace_hook()
    tmpdir = os.environ.get("KERNEL_TRACE_DIR") or None
    if tmpdir:
        os.makedirs(tmpdir, exist_ok=True)
    res = run_bass_kernel_spmd(
        nc, in_maps, core_ids=list(range(N_CORES)), trace=trace, tmpdir=tmpdir
    )
    kernel.last_exec_time_ns = res.exec_time_ns

    out = np.empty((N_FULL, COUT, HO, WO), dtype=np.float32)
    for c in range(N_CORES):
        o = res.results[c]["out"]  # [4, 2, 128, 2916]
        out[c * N_PER_CORE:(c + 1) * N_PER_CORE] = o.reshape(N_PER_CORE, COUT, HO, WO)
    return out


kernel.last_exec_time_ns = None


# revision 22
# speedup vs baseline: 1.0508x; 1.0508x over previous
"""Trainium2 Bass kernel for quantized int8 Conv2d (dequant -> conv -> bias).

Reference computation (per-tensor quant):
    x = (inputVec.f32 - 7) * 0.01        # [32, 128, 56, 56]
    w = (weight.f32   - 3) * 0.01        # [256, 128, 3, 3]
    b = clip(round(bias / 1e-4)) * 1e-4  # [256]
    out = conv2d(x, w, VALID) + b        # [32, 256, 54, 54] f32

Strategy:
  - Data-parallel over batch: 32 images -> 8 cores x 4 images.
  - Integer arithmetic is exact in bf16/fp32: (w-3) in [-131,124] and x_q in
    [-128,127] are exactly representable in bf16; the 1152-term dot products
    stay well inside fp32's exact-integer range in practice.
  - Because padding is VALID, every output pixel sees the full 3x3x128 window,
    so the input zero-point (-7) contribution is a per-channel constant:
        sum (x-7)(w-3) = sum x*(w-3) - 7 * S3[co],  S3[co] = sum (w-3)
    We fold -7e-4*S3 plus the quantized bias into one per-channel bias on host.
  - On device: conv as 9 shifted matmuls (kh,kw) accumulating in PSUM.
    lhsT = (w-3)[ci, co_chunk] bf16 (128x128), rhs = x_bf16[ci, 9 rows x 54
    cols strided view] (N=486 <= 512 one PSUM bank), out chunks of 128 Cout.
  - Epilogue on ScalarE: out = psum * 1e-4 + bias_eff (per-partition bias AP),
    then DMA to DRAM.
"""

import os
import sys

import numpy as np

if "/opt/trn_rl_repo" not in sys.path:
    sys.path.insert(0, "/opt/trn_rl_repo")

import ml_dtypes

import concourse.bass as bass
import concourse.mybir as mybir
from concourse import bacc
from concourse.tile import TileContext

# Problem constants (hardcoded per harness contract)
N_FULL = 32
CIN = 128
H = 56
W = 56
COUT = 256
KH = 3
KW = 3
HO = H - 2  # 54
WO = W - 2  # 54
N_CORES = 8
N_PER_CORE = N_FULL // N_CORES  # 4

IN_SCALE, IN_ZP = 0.01, 7
W_SCALE, W_ZP = 0.01, 3
B_SCALE = 1e-4
OUT_SCALE = IN_SCALE * W_SCALE  # 1e-4

ROWS_PER_TILE = 9           # output rows per matmul group
N_ROW_TILES = HO // ROWS_PER_TILE  # 6
NFREE = ROWS_PER_TILE * WO  # 486 <= 512 (one PSUM bank)


def build_nc():
    nc = bacc.Bacc()

    x = nc.declare_dram_parameter("x", [N_PER_CORE, CIN, H * W], mybir.dt.int8, isOutput=False)
    wT = nc.declare_dram_parameter("wT", [CIN, KH * KW * COUT], mybir.dt.bfloat16, isOutput=False)
    b = nc.declare_dram_parameter("b", [CIN, 2], mybir.dt.float32, isOutput=False)
    out = nc.declare_dram_parameter("out", [N_PER_CORE, 2, 128, HO * WO], mybir.dt.float32, isOutput=True)

    with TileContext(nc) as tc:
        with (
            tc.tile_pool(name="const", bufs=1) as const_pool,
            tc.tile_pool(name="xq", bufs=4) as xq_pool,
            tc.tile_pool(name="xb", bufs=4) as xb_pool,
            tc.tile_pool(name="psum", bufs=6, space="PSUM") as psum_pool,
            tc.tile_pool(name="osb", bufs=8) as out_pool,
        ):
            # Warm up the PE's HAM clock-gate with dummy matmuls on a memset
            # tile so the first real matmuls run at 2.4 GHz, overlapping the
            # input DMA + cast latency.
            warm = const_pool.tile([128, 512], mybir.dt.bfloat16)
            nc.vector.memset(warm[:], 0)
            warm_ps = psum_pool.tile([128, 512], mybir.dt.float32, tag="warm", bufs=1)
            for _ in range(10):
                nc.tensor.matmul(warm_ps[:], lhsT=warm[:, :128], rhs=warm[:],
                                 start=True, stop=True)

            # Input images on the SP HWDGE queue, in half-image chunks so the
            # int8->bf16 cast overlaps the DMA; weights/bias on the ACT HWDGE
            # queue so they don't serialize in front of image 0.
            HALF = H * W // 2
            xqs, xbs = [], []
            for n in range(N_PER_CORE):
                xq = xq_pool.tile([CIN, H * W], mybir.dt.int8, name=f"xq{n}", tag="xq")
                nc.sync.dma_start(out=xq[:, :HALF], in_=x[n, :, :HALF])
                nc.sync.dma_start(out=xq[:, HALF:], in_=x[n, :, HALF:])
                xqs.append(xq)

            # Weights: [ci, (k, co)] bf16, all 9 taps in one contiguous DMA.
            w_sb = const_pool.tile([CIN, KH * KW * COUT], mybir.dt.bfloat16)
            nc.scalar.dma_start(out=w_sb[:], in_=wT[:])
            # Effective bias: [co_within_chunk, chunk] f32
            b_sb = const_pool.tile([128, 2], mybir.dt.float32)
            nc.scalar.dma_start(out=b_sb[:], in_=b[:])

            for n in range(N_PER_CORE):
                xb = xb_pool.tile([CIN, H * W], mybir.dt.bfloat16, name=f"xb{n}", tag="xb")
                nc.vector.tensor_copy(xb[:, :HALF], xqs[n][:, :HALF])
                nc.vector.tensor_copy(xb[:, HALF:], xqs[n][:, HALF:])
                xbs.append(xb)

            for n in range(N_PER_CORE):
                xb3 = xbs[n].rearrange("p (h w) -> p h w", w=W)

                for chunk in range(2):
                    ot = out_pool.tile([128, HO * WO], mybir.dt.float32)
                    for rt in range(N_ROW_TILES):
                        ps = psum_pool.tile([128, NFREE], mybir.dt.float32)
                        y0 = rt * ROWS_PER_TILE
                        for k in range(KH * KW):
                            kh, kw = divmod(k, KW)
                            lhsT = w_sb[:, k * COUT + chunk * 128:k * COUT + chunk * 128 + 128]
                            rhs = xb3[:, y0 + kh:y0 + kh + ROWS_PER_TILE, kw:kw + WO]
                            nc.tensor.matmul(
                                ps[:], lhsT=lhsT, rhs=rhs,
                                start=(k == 0), stop=(k == KH * KW - 1),
                            )
                        nc.scalar.activation(
                            ot[:, y0 * WO:(y0 + ROWS_PER_TILE) * WO], ps[:],
                            mybir.ActivationFunctionType.Identity,
                            bias=b_sb[:, chunk:chunk + 1], scale=float(OUT_SCALE),
                        )
                        if rt % 2 == 1:  # store every 2 row tiles for overlap
                            c0 = (y0 - ROWS_PER_TILE) * WO
                            c1 = (y0 + ROWS_PER_TILE) * WO
                            nc.sync.dma_start(
                                out=out[n, chunk, :, c0:c1], in_=ot[:, c0:c1]
                            )
    nc.compile()
    return nc


def host_prep(weight: np.ndarray, bias: np.ndarray):
    """Prepare the replicated weight/bias operands (layout + zero-point folding)."""
    w_int = weight.astype(np.int32) - W_ZP  # [256, 128, 3, 3], in [-131, 124]
    # lhsT layout: [ci, (kh*kw, co)] so the whole thing is one contiguous DMA
    wT = np.ascontiguousarray(
        w_int.transpose(1, 2, 3, 0).reshape(CIN, KH * KW * COUT)
    )
    wT_bf16 = wT.astype(ml_dtypes.bfloat16)  # exact (|v| <= 131 < 256)

    # Quantized bias, matching reference rounding (f32 ops, round-half-even)
    b_q = np.clip(np.round(bias.astype(np.float32) / np.float32(B_SCALE)),
                  -(2.0 ** 31), 2.0 ** 31 - 1).astype(np.float32) * np.float32(B_SCALE)
    s3 = w_int.reshape(COUT, -1).sum(axis=1).astype(np.float64)  # sum (w-3) per co
    b_eff = (b_q.astype(np.float64) - IN_ZP * OUT_SCALE * s3).astype(np.float32)
    # [co_within_chunk, chunk] so the SBUF partition dim is contiguous in DRAM
    b_dev = np.ascontiguousarray(b_eff.reshape(2, 128).T)
    return wT_bf16, b_dev


_CACHED_NC = None


def _install_trace_hook():
    """Register the axon NTFF profile hook that this container's antenv stub
    lacks, and neutralize the remote artifact upload. Best-effort."""
    try:
        import types

        import antenv  # noqa: F401

        if "antenv.axon_hooks" not in sys.modules:
            from trn_agent_boot.trn_boot import _ntff_profile_via_ctypes

            hook = _ntff_profile_via_ctypes("/opt/axon/libaxon_pjrt.so")
            mod = types.ModuleType("antenv.axon_hooks")
            mod.get_axon_ntff_profile_hook = lambda: hook
            mod.set_axon_ntff_profile_hook = lambda h: None
            sys.modules["antenv.axon_hooks"] = mod
        import concourse.bass_utils as bu

        bu.upload_artifacts = lambda tmpdir: str(tmpdir)
        return True
    except Exception as e:  # pragma: no cover - profiling is best-effort
        print(f"trace hook unavailable: {type(e).__name__}: {e}", file=sys.stderr)
        return False


def kernel(inputVec: np.ndarray, weight: np.ndarray, bias: np.ndarray) -> np.ndarray:
    global _CACHED_NC
    from concourse.bass_utils import run_bass_kernel_spmd

    wT_bf16, b_dev = host_prep(weight, bias)
    x = np.ascontiguousarray(inputVec.reshape(N_FULL, CIN, H * W))

    if _CACHED_NC is None:
        _CACHED_NC = build_nc()
    nc = _CACHED_NC

    in_maps = []
    for c in range(N_CORES):
        in_maps.append({
            "x": x[c * N_PER_CORE:(c + 1) * N_PER_CORE],
            "wT": wT_bf16,
            "b": b_dev,
        })

    trace = os.environ.get("KERNEL_TRACE", "0") == "1"
    if trace:
        trace = _install_trace_hook()
    tmpdir = os.environ.get("KERNEL_TRACE_DIR") or None
    if tmpdir:
        os.makedirs(tmpdir, exist_ok=True)
    res = run_bass_kernel_spmd(
        nc, in_maps, core_ids=list(range(N_CORES)), trace=trace, tmpdir=tmpdir
    )
    kernel.last_exec_time_ns = res.exec_time_ns

    out = np.empty((N_FULL, COUT, HO, WO), dtype=np.float32)
    for c in range(N_CORES):
        o = res.results[c]["out"]  # [4, 2, 128, 2916]
        out[c * N_PER_CORE:(c + 1) * N_PER_CORE] = o.reshape(N_PER_CORE, COUT, HO, WO)
    return out


kernel.last_exec_time_ns = None
